# revision 6
# baseline (speedup 1.0000x reference)
"""CompressK: mean-pool overlapping (size=32, stride=16) token chunks of
k[total_tokens, H, D] per ragged sequence, on 8 NeuronCores via Bass/Tile.

Math: with stride 16 and size 32, chunk c of a sequence is
    (blocksum[c] + blocksum[c+1]) / 32
where blocksum[b] is the sum of 16-token block b. Each input byte is read
once (plus ~1% tile-overlap), so the kernel is HBM-bound as intended.

Per-core layout: partition = 16-token block (contiguous 32KB in DRAM),
free dim = 16 tokens x H*D = 8192 f32. Free-dim tree-adds produce block
sums [128, 512]; the cross-partition neighbor add uses a small SBUF->SBUF
DMA to shift partitions by one (compute engines cannot read
partition-shifted operands: AP partition starts must be 0/32/64/96, and
the fp32 matmul alternative trips a codegen sync-wait limit).

Sharding: all chunks are split into 8 near-equal contiguous runs; each
core's blocks are host-sliced into a dense [B, 8192] buffer (segments of
different sequences concatenated, one garbage chunk slot per boundary,
dropped on gather). SPMD program is identical across cores.
"""

import numpy as np

_NCORES = 8
_KS = 32  # kernel_size
_ST = 16  # kernel_stride
_HD = 512  # H * D
_BLK = _ST * _HD  # elems per 16-token block row

_nc_cache: dict = {}


def _plan(cu, n_cores=_NCORES):
    """Split all chunks into n_cores contiguous runs; per core a list of
    (seq, first_chunk_in_seq, n_chunks) segments."""
    cu = np.asarray(cu, dtype=np.int64)
    lens = cu[1:] - cu[:-1]
    nch = np.maximum(0, (lens - _KS) // _ST + 1)
    cu_comp = np.zeros(len(cu), np.int32)
    cu_comp[1:] = np.cumsum(nch)
    n_total = int(cu_comp[-1])
    base, rem = divmod(n_total, n_cores)
    counts = [base + (1 if i < rem else 0) for i in range(n_cores)]
    bounds = np.concatenate([[0], np.cumsum(counts)])
    jobs = []
    for i in range(n_cores):
        g0, g1 = int(bounds[i]), int(bounds[i + 1])
        segs = []
        for s in range(len(lens)):
            lo = max(g0, int(cu_comp[s]))
            hi = min(g1, int(cu_comp[s + 1]))
            if lo < hi:
                segs.append((s, lo - int(cu_comp[s]), hi - lo))
        jobs.append(segs)
    return jobs, cu_comp, n_total


def _build_bass(n_blocks):
    """Bass program: k[n_blocks, 8192] -> out[n_blocks-1, 512] where
    out[i] = (sum(k[i]) + sum(k[i+1])) / 32 over 16-token groups."""
    import concourse.bacc as bacc
    import concourse.mybir as mybir
    from concourse.tile import TileContext

    f32 = mybir.dt.float32
    nc = bacc.Bacc()
    kin = nc.dram_tensor("k", [n_blocks, _BLK], f32, kind="ExternalInput")
    out = nc.dram_tensor("out", [n_blocks - 1, _HD], f32, kind="ExternalOutput")

    with TileContext(nc) as tc:
        with (
            tc.tile_pool(name="kin", bufs=3) as kin_pool,
            tc.tile_pool(name="tree", bufs=2) as tree_pool,
            tc.tile_pool(name="bs", bufs=2) as bs_pool,
            tc.tile_pool(name="outp", bufs=2) as out_pool,
        ):
            for t0 in range(0, n_blocks - 1, 127):
                nch = min(127, n_blocks - 1 - t0)
                nb = nch + 1
                kt = kin_pool.tile([128, _BLK], f32, tag="kt")
                nc.sync.dma_start(out=kt[:nb], in_=kin[t0 : t0 + nb, :])
                s1 = tree_pool.tile([128, _BLK // 2], f32, tag="s1")
                nc.vector.tensor_add(s1[:nb], kt[:nb, : _BLK // 2], kt[:nb, _BLK // 2 :])
                s2 = tree_pool.tile([128, _BLK // 4], f32, tag="s2")
                nc.vector.tensor_add(s2[:nb], s1[:nb, : _BLK // 4], s1[:nb, _BLK // 4 :])
                s3 = tree_pool.tile([128, _BLK // 8], f32, tag="s3")
                nc.vector.tensor_add(s3[:nb], s2[:nb, : _BLK // 8], s2[:nb, _BLK // 8 :])
                bs = bs_pool.tile([128, _HD], f32, tag="bs")
                nc.vector.tensor_add(bs[:nb], s3[:nb, :_HD], s3[:nb, _HD:])
                bsh = bs_pool.tile([128, _HD], f32, tag="bsh")
                nc.sync.dma_start(out=bsh[:nch], in_=bs[1:nb])
                fs = bs_pool.tile([128, _HD], f32, tag="fs")
                nc.vector.tensor_add(fs[:nch], bs[:nch], bsh[:nch])
                ot = out_pool.tile([128, _HD], f32, tag="ot")
                nc.scalar.mul(ot[:nch], fs[:nch], 1.0 / _KS)
                nc.sync.dma_start(out=out[t0 : t0 + nch, :], in_=ot[:nch])
    nc.finalize()
    return nc


def _numpy_fallback(k, cu, kernel_size, kernel_stride):
    cu = np.asarray(cu, dtype=np.int64)
    lens = cu[1:] - cu[:-1]
    max_c = max(0, (int(lens.max()) - kernel_size) // kernel_stride + 1)
    offs = np.arange(max_c, dtype=np.int64) * kernel_stride
    starts = cu[:-1, None] + offs[None, :]
    valid = (offs[None, :] + kernel_size) <= lens[:, None]
    vstarts = starts[valid]
    idx = vstarts[:, None] + np.arange(kernel_size, dtype=np.int64)[None, :]
    cu_comp = np.zeros(len(cu), np.int32)
    cu_comp[1:] = np.cumsum(valid.sum(axis=1))
    return k[idx].mean(axis=1), cu_comp


def kernel(k, cu_seqlens, kernel_size, kernel_stride):
    k = np.asarray(k)
    cu = np.asarray(cu_seqlens)
    ks = int(np.asarray(kernel_size))
    st = int(np.asarray(kernel_stride))
    T, H, D = k.shape

    if ks != _KS or st != _ST or H * D != _HD or k.dtype != np.float32:
        out, cu_comp = _numpy_fallback(k, cu, ks, st)
        return out.astype(k.dtype), cu_comp

    from concourse.bass_utils import run_bass_kernel_spmd

    jobs, cu_comp, n_total = _plan(cu)

    kflat = np.ascontiguousarray(k).reshape(T, _HD)
    per_core_rows = []
    per_core_slots = []  # (local chunk-slot indices, global chunk ids)
    cu64 = cu.astype(np.int64)
    for segs in jobs:
        rows = []
        slots, gids = [], []
        b = 0
        for s, c0, n in segs:
            t0 = int(cu64[s]) + c0 * _ST
            rows.append(kflat[t0 : t0 + (n + 1) * _ST].reshape(n + 1, _BLK))
            slots.extend(range(b, b + n))
            gids.extend(range(int(cu_comp[s]) + c0, int(cu_comp[s]) + c0 + n))
            b += n + 1
        per_core_rows.append(rows)
        per_core_slots.append((np.asarray(slots, np.int64), np.asarray(gids, np.int64)))

    n_blocks = max(sum(r.shape[0] for r in rows) for rows in per_core_rows)
    in_k = []
    for rows in per_core_rows:
        buf = np.zeros((n_blocks, _BLK), np.float32)
        b = 0
        for r in rows:
            buf[b : b + r.shape[0]] = r
            b += r.shape[0]
        in_k.append(buf)

    if n_blocks not in _nc_cache:
        _nc_cache[n_blocks] = _build_bass(n_blocks)
    nc = _nc_cache[n_blocks]

    in_maps = [{"k": in_k[c]} for c in range(_NCORES)]
    results = run_bass_kernel_spmd(nc, in_maps, list(range(_NCORES))).results

    full = np.empty((n_total, _HD), np.float32)
    for c in range(_NCORES):
        slots, gids = per_core_slots[c]
        if len(slots):
            full[gids] = results[c]["out"][slots]
    return full.reshape(n_total, H, D), cu_comp


# revision 10
# speedup vs baseline: 1.4974x; 1.4974x over previous
"""CompressK: mean-pool overlapping (size=32, stride=16) token chunks of
k[total_tokens, H, D] per ragged sequence, on 8 NeuronCores via Bass/Tile.

Math: with stride 16 and size 32, chunk c of a sequence is
    (blocksum[c] + blocksum[c+1]) / 32
where blocksum[b] is the sum of 16-token block b. Each input byte is read
once (plus ~1% tile-overlap), so the kernel is HBM-bound as intended.

Per-core layout: partition = 16-token block (contiguous 32KB in DRAM),
free dim = 16 tokens x H*D = 8192 f32. Free-dim tree-adds produce block
sums [128, 512]; the cross-partition neighbor add runs on the (otherwise
idle) TensorEngine as a banded [K=128, M=127] fp32 matmul (compute
engines cannot read partition-shifted operands: AP partition starts must
be 0/32/64/96). PSUM -> SBUF scale-copy on ACT, then store.

DMA ring discipline: input loads issue on the SP (sync) HWDGE ring,
output stores on the Activation ring. HWDGE rings are FIFO per issuing
engine, so putting compute-dependent stores on the load ring would stall
tile i+1's load behind tile i's whole compute chain.

Sharding: all chunks are split into 8 near-equal contiguous runs; each
core's blocks are host-sliced into a dense [B, 8192] buffer (segments of
different sequences concatenated, one garbage chunk slot per boundary,
dropped on gather). SPMD program is identical across cores.
"""

import numpy as np

_NCORES = 8
_KS = 32  # kernel_size
_ST = 16  # kernel_stride
_HD = 512  # H * D
_BLK = _ST * _HD  # elems per 16-token block row

_nc_cache: dict = {}


def _plan(cu, n_cores=_NCORES):
    """Split all chunks into n_cores contiguous runs; per core a list of
    (seq, first_chunk_in_seq, n_chunks) segments."""
    cu = np.asarray(cu, dtype=np.int64)
    lens = cu[1:] - cu[:-1]
    nch = np.maximum(0, (lens - _KS) // _ST + 1)
    cu_comp = np.zeros(len(cu), np.int32)
    cu_comp[1:] = np.cumsum(nch)
    n_total = int(cu_comp[-1])
    base, rem = divmod(n_total, n_cores)
    counts = [base + (1 if i < rem else 0) for i in range(n_cores)]
    bounds = np.concatenate([[0], np.cumsum(counts)])
    jobs = []
    for i in range(n_cores):
        g0, g1 = int(bounds[i]), int(bounds[i + 1])
        segs = []
        for s in range(len(lens)):
            lo = max(g0, int(cu_comp[s]))
            hi = min(g1, int(cu_comp[s + 1]))
            if lo < hi:
                segs.append((s, lo - int(cu_comp[s]), hi - lo))
        jobs.append(segs)
    return jobs, cu_comp, n_total


def _build_bass(n_blocks):
    """Bass program: k[n_blocks, 8192] -> out[n_blocks-1, 512] where
    out[i] = (sum(k[i]) + sum(k[i+1])) / 32 over 16-token groups."""
    import concourse.bacc as bacc
    import concourse.mybir as mybir
    from concourse.tile import TileContext

    f32 = mybir.dt.float32
    nc = bacc.Bacc()
    kin = nc.dram_tensor("k", [n_blocks, _BLK], f32, kind="ExternalInput")
    win = nc.dram_tensor("w", [128, 127], f32, kind="ExternalInput")
    out = nc.dram_tensor("out", [n_blocks - 1, _HD], f32, kind="ExternalOutput")

    with TileContext(nc) as tc:
        with (
            tc.tile_pool(name="wpool", bufs=1) as wpool,
            tc.tile_pool(name="kin", bufs=3) as kin_pool,
            tc.tile_pool(name="tree", bufs=2) as tree_pool,
            tc.tile_pool(name="bs", bufs=2) as bs_pool,
            tc.tile_pool(name="outp", bufs=2) as out_pool,
            tc.tile_pool(name="psum", bufs=2, space="PSUM") as psum_pool,
        ):
            w = wpool.tile([128, 127], f32)
            nc.sync.dma_start(out=w[:, :], in_=win[:, :])
            for t0 in range(0, n_blocks - 1, 127):
                nch = min(127, n_blocks - 1 - t0)
                nb = nch + 1
                kt = kin_pool.tile([128, _BLK], f32, tag="kt")
                nc.sync.dma_start(out=kt[:nb], in_=kin[t0 : t0 + nb, :])
                s1 = tree_pool.tile([128, _BLK // 2], f32, tag="s1")
                nc.vector.tensor_add(s1[:nb], kt[:nb, : _BLK // 2], kt[:nb, _BLK // 2 :])
                s2 = tree_pool.tile([128, _BLK // 4], f32, tag="s2")
                nc.vector.tensor_add(s2[:nb], s1[:nb, : _BLK // 4], s1[:nb, _BLK // 4 :])
                s3 = tree_pool.tile([128, _BLK // 8], f32, tag="s3")
                nc.vector.tensor_add(s3[:nb], s2[:nb, : _BLK // 8], s2[:nb, _BLK // 8 :])
                bs = bs_pool.tile([128, _HD], f32, tag="bs")
                nc.vector.tensor_add(bs[:nb], s3[:nb, :_HD], s3[:nb, _HD:])
                ps = psum_pool.tile([127, _HD], f32, tag="ps")
                nc.tensor.matmul(ps[:nch], w[:nb, :nch], bs[:nb], start=True, stop=True)
                ot = out_pool.tile([128, _HD], f32, tag="ot")
                nc.scalar.mul(ot[:nch], ps[:nch], 1.0 / _KS)
                nc.scalar.dma_start(out=out[t0 : t0 + nch, :], in_=ot[:nch])
    nc.finalize()
    return nc


def _numpy_fallback(k, cu, kernel_size, kernel_stride):
    cu = np.asarray(cu, dtype=np.int64)
    lens = cu[1:] - cu[:-1]
    max_c = max(0, (int(lens.max()) - kernel_size) // kernel_stride + 1)
    offs = np.arange(max_c, dtype=np.int64) * kernel_stride
    starts = cu[:-1, None] + offs[None, :]
    valid = (offs[None, :] + kernel_size) <= lens[:, None]
    vstarts = starts[valid]
    idx = vstarts[:, None] + np.arange(kernel_size, dtype=np.int64)[None, :]
    cu_comp = np.zeros(len(cu), np.int32)
    cu_comp[1:] = np.cumsum(valid.sum(axis=1))
    return k[idx].mean(axis=1), cu_comp


def kernel(k, cu_seqlens, kernel_size, kernel_stride):
    k = np.asarray(k)
    cu = np.asarray(cu_seqlens)
    ks = int(np.asarray(kernel_size))
    st = int(np.asarray(kernel_stride))
    T, H, D = k.shape

    if ks != _KS or st != _ST or H * D != _HD or k.dtype != np.float32:
        out, cu_comp = _numpy_fallback(k, cu, ks, st)
        return out.astype(k.dtype), cu_comp

    from concourse.bass_utils import run_bass_kernel_spmd

    jobs, cu_comp, n_total = _plan(cu)

    kflat = np.ascontiguousarray(k).reshape(T, _HD)
    per_core_rows = []
    per_core_slots = []  # (local chunk-slot indices, global chunk ids)
    cu64 = cu.astype(np.int64)
    for segs in jobs:
        rows = []
        slots, gids = [], []
        b = 0
        for s, c0, n in segs:
            t0 = int(cu64[s]) + c0 * _ST
            rows.append(kflat[t0 : t0 + (n + 1) * _ST].reshape(n + 1, _BLK))
            slots.extend(range(b, b + n))
            gids.extend(range(int(cu_comp[s]) + c0, int(cu_comp[s]) + c0 + n))
            b += n + 1
        per_core_rows.append(rows)
        per_core_slots.append((np.asarray(slots, np.int64), np.asarray(gids, np.int64)))

    n_blocks = max(sum(r.shape[0] for r in rows) for rows in per_core_rows)
    in_k = []
    for rows in per_core_rows:
        buf = np.zeros((n_blocks, _BLK), np.float32)
        b = 0
        for r in rows:
            buf[b : b + r.shape[0]] = r
            b += r.shape[0]
        in_k.append(buf)

    wb = np.zeros((128, 127), np.float32)
    ii = np.arange(127)
    wb[ii, ii] = 1.0
    wb[ii + 1, ii] = 1.0

    if n_blocks not in _nc_cache:
        _nc_cache[n_blocks] = _build_bass(n_blocks)
    nc = _nc_cache[n_blocks]

    in_maps = [{"k": in_k[c], "w": wb} for c in range(_NCORES)]
    results = run_bass_kernel_spmd(nc, in_maps, list(range(_NCORES))).results

    full = np.empty((n_total, _HD), np.float32)
    for c in range(_NCORES):
        slots, gids = per_core_slots[c]
        if len(slots):
            full[gids] = results[c]["out"][slots]
    return full.reshape(n_total, H, D), cu_comp


# revision 11
# speedup vs baseline: 1.5269x; 1.0197x over previous
"""CompressK: mean-pool overlapping (size=32, stride=16) token chunks of
k[total_tokens, H, D] per ragged sequence, on 8 NeuronCores via Bass/Tile.

Math: with stride 16 and size 32, chunk c of a sequence is
    (blocksum[c] + blocksum[c+1]) / 32
where blocksum[b] is the sum of 16-token block b. Each input byte is read
once (plus ~1% tile-overlap), so the kernel is HBM-bound as intended.

Per-core layout: partition = 16-token block (contiguous 32KB in DRAM),
free dim = 16 tokens x H*D = 8192 f32. Free-dim tree-adds produce block
sums [128, 512]; the cross-partition neighbor add runs on the (otherwise
idle) TensorEngine as a banded [K=128, M=127] fp32 matmul (compute
engines cannot read partition-shifted operands: AP partition starts must
be 0/32/64/96). PSUM -> SBUF scale-copy on ACT, then store.

DMA ring discipline: input loads issue on the SP (sync) HWDGE ring,
output stores on the Activation ring. HWDGE rings are FIFO per issuing
engine, so putting compute-dependent stores on the load ring would stall
tile i+1's load behind tile i's whole compute chain.

Sharding: all chunks are split into 8 near-equal contiguous runs; each
core's blocks are host-sliced into a dense [B, 8192] buffer (segments of
different sequences concatenated, one garbage chunk slot per boundary,
dropped on gather). SPMD program is identical across cores.
"""

import numpy as np

_NCORES = 8
_KS = 32  # kernel_size
_ST = 16  # kernel_stride
_HD = 512  # H * D
_BLK = _ST * _HD  # elems per 16-token block row

_nc_cache: dict = {}


def _plan(cu, n_cores=_NCORES):
    """Split all chunks into n_cores contiguous runs; per core a list of
    (seq, first_chunk_in_seq, n_chunks) segments."""
    cu = np.asarray(cu, dtype=np.int64)
    lens = cu[1:] - cu[:-1]
    nch = np.maximum(0, (lens - _KS) // _ST + 1)
    cu_comp = np.zeros(len(cu), np.int32)
    cu_comp[1:] = np.cumsum(nch)
    n_total = int(cu_comp[-1])
    base, rem = divmod(n_total, n_cores)
    counts = [base + (1 if i < rem else 0) for i in range(n_cores)]
    bounds = np.concatenate([[0], np.cumsum(counts)])
    jobs = []
    for i in range(n_cores):
        g0, g1 = int(bounds[i]), int(bounds[i + 1])
        segs = []
        for s in range(len(lens)):
            lo = max(g0, int(cu_comp[s]))
            hi = min(g1, int(cu_comp[s + 1]))
            if lo < hi:
                segs.append((s, lo - int(cu_comp[s]), hi - lo))
        jobs.append(segs)
    return jobs, cu_comp, n_total


def _build_bass(n_blocks):
    """Bass program: k[n_blocks, 8192] -> out[n_blocks-1, 512] where
    out[i] = (sum(k[i]) + sum(k[i+1])) / 32 over 16-token groups."""
    import concourse.bacc as bacc
    import concourse.mybir as mybir
    from concourse.tile import TileContext

    f32 = mybir.dt.float32
    nc = bacc.Bacc()
    kin = nc.dram_tensor("k", [n_blocks, _BLK], f32, kind="ExternalInput")
    win = nc.dram_tensor("w", [128, 127], f32, kind="ExternalInput")
    out = nc.dram_tensor("out", [n_blocks - 1, _HD], f32, kind="ExternalOutput")

    qb = _BLK // 4  # quarter of a block row: 4 tokens x 512 = 2048 f32
    with TileContext(nc) as tc:
        with (
            tc.tile_pool(name="wpool", bufs=1) as wpool,
            tc.tile_pool(name="kin", bufs=8) as kin_pool,
            tc.tile_pool(name="tree", bufs=3) as tree_pool,
            tc.tile_pool(name="bq", bufs=6) as bq_pool,
            tc.tile_pool(name="bs", bufs=2) as bs_pool,
            tc.tile_pool(name="outp", bufs=2) as out_pool,
            tc.tile_pool(name="psum", bufs=2, space="PSUM") as psum_pool,
        ):
            w = wpool.tile([128, 127], f32)
            nc.sync.dma_start(out=w[:, :], in_=win[:, :])
            for t0 in range(0, n_blocks - 1, 127):
                nch = min(127, n_blocks - 1 - t0)
                nb = nch + 1
                # per quarter: load [nb, 2048] (4 tokens), tree to [nb, 512]
                bq = []
                for q in range(4):
                    ktq = kin_pool.tile([128, qb], f32, tag="ktq")
                    nc.sync.dma_start(
                        out=ktq[:nb], in_=kin[t0 : t0 + nb, q * qb : (q + 1) * qb]
                    )
                    aq = tree_pool.tile([128, qb // 2], f32, tag="aq")
                    nc.vector.tensor_add(
                        aq[:nb], ktq[:nb, : qb // 2], ktq[:nb, qb // 2 :]
                    )
                    b = bq_pool.tile([128, _HD], f32, tag=f"bq{q}")
                    nc.vector.tensor_add(b[:nb], aq[:nb, :_HD], aq[:nb, _HD:])
                    bq.append(b)
                c01 = bq_pool.tile([128, _HD], f32, tag="c01")
                nc.vector.tensor_add(c01[:nb], bq[0][:nb], bq[1][:nb])
                c23 = bq_pool.tile([128, _HD], f32, tag="c23")
                nc.vector.tensor_add(c23[:nb], bq[2][:nb], bq[3][:nb])
                bs = bs_pool.tile([128, _HD], f32, tag="bs")
                nc.vector.tensor_add(bs[:nb], c01[:nb], c23[:nb])
                ps = psum_pool.tile([127, _HD], f32, tag="ps")
                nc.tensor.matmul(ps[:nch], w[:nb, :nch], bs[:nb], start=True, stop=True)
                ot = out_pool.tile([128, _HD], f32, tag="ot")
                nc.scalar.mul(ot[:nch], ps[:nch], 1.0 / _KS)
                nc.scalar.dma_start(out=out[t0 : t0 + nch, :], in_=ot[:nch])
    nc.finalize()
    return nc


def _numpy_fallback(k, cu, kernel_size, kernel_stride):
    cu = np.asarray(cu, dtype=np.int64)
    lens = cu[1:] - cu[:-1]
    max_c = max(0, (int(lens.max()) - kernel_size) // kernel_stride + 1)
    offs = np.arange(max_c, dtype=np.int64) * kernel_stride
    starts = cu[:-1, None] + offs[None, :]
    valid = (offs[None, :] + kernel_size) <= lens[:, None]
    vstarts = starts[valid]
    idx = vstarts[:, None] + np.arange(kernel_size, dtype=np.int64)[None, :]
    cu_comp = np.zeros(len(cu), np.int32)
    cu_comp[1:] = np.cumsum(valid.sum(axis=1))
    return k[idx].mean(axis=1), cu_comp


def kernel(k, cu_seqlens, kernel_size, kernel_stride):
    k = np.asarray(k)
    cu = np.asarray(cu_seqlens)
    ks = int(np.asarray(kernel_size))
    st = int(np.asarray(kernel_stride))
    T, H, D = k.shape

    if ks != _KS or st != _ST or H * D != _HD or k.dtype != np.float32:
        out, cu_comp = _numpy_fallback(k, cu, ks, st)
        return out.astype(k.dtype), cu_comp

    from concourse.bass_utils import run_bass_kernel_spmd

    jobs, cu_comp, n_total = _plan(cu)

    kflat = np.ascontiguousarray(k).reshape(T, _HD)
    per_core_rows = []
    per_core_slots = []  # (local chunk-slot indices, global chunk ids)
    cu64 = cu.astype(np.int64)
    for segs in jobs:
        rows = []
        slots, gids = [], []
        b = 0
        for s, c0, n in segs:
            t0 = int(cu64[s]) + c0 * _ST
            rows.append(kflat[t0 : t0 + (n + 1) * _ST].reshape(n + 1, _BLK))
            slots.extend(range(b, b + n))
            gids.extend(range(int(cu_comp[s]) + c0, int(cu_comp[s]) + c0 + n))
            b += n + 1
        per_core_rows.append(rows)
        per_core_slots.append((np.asarray(slots, np.int64), np.asarray(gids, np.int64)))

    n_blocks = max(sum(r.shape[0] for r in rows) for rows in per_core_rows)
    in_k = []
    for rows in per_core_rows:
        buf = np.zeros((n_blocks, _BLK), np.float32)
        b = 0
        for r in rows:
            buf[b : b + r.shape[0]] = r
            b += r.shape[0]
        in_k.append(buf)

    wb = np.zeros((128, 127), np.float32)
    ii = np.arange(127)
    wb[ii, ii] = 1.0
    wb[ii + 1, ii] = 1.0

    if n_blocks not in _nc_cache:
        _nc_cache[n_blocks] = _build_bass(n_blocks)
    nc = _nc_cache[n_blocks]

    in_maps = [{"k": in_k[c], "w": wb} for c in range(_NCORES)]
    results = run_bass_kernel_spmd(nc, in_maps, list(range(_NCORES))).results

    full = np.empty((n_total, _HD), np.float32)
    for c in range(_NCORES):
        slots, gids = per_core_slots[c]
        if len(slots):
            full[gids] = results[c]["out"][slots]
    return full.reshape(n_total, H, D), cu_comp
